# revision 5
# baseline (speedup 1.0000x reference)
"""Trainium2 Bass kernel v2 for nn_CausalSelfAttention_57861799412149.

Same sequence-parallel sharding as v1 (8 cores = 2 batches x 4 query chunks of
512; per-core 1536-key window). Redesigned to minimize instruction count for
the simulator-like cost model (~60-70us/instruction + ~0.9us/MF f32r matmul +
~34us/MB DMA; bf16 matmuls are ~5x/MF more expensive -> all matmuls f32r):

- One wide DMA per tensor (weights/x/ve concatenated along free dim on host).
- Gates: 12 matmuls into one [128,48] PSUM tile + 1 tanh + 1 add (was 36).
- K/Q projections into multi-bank PSUM tiles ([128,3072]/[128,4096]) followed
  by ONE wide rope+rmsnorm chain (15 instrs) using gpsimd partition-reduce /
  partition-broadcast (all gpsimd outputs at partition offset 0).
- Attention: S computed full-width (w=512 always) so exp runs once per THREE
  key blocks ([128,3072] PSUM -> SBUF); PV matmuls band-clipped f32r (no
  bf16, no ldweights); masks only on the two boundary diagonals.
"""
import sys

sys.path.insert(0, "/opt/trn_rl_repo")

import numpy as np

import concourse.bass as bass
import concourse.tile as tile
from concourse import bacc, mybir
from concourse.tile import add_dep_helper

B, T, NE = 2, 2048, 1024
NH, NKV, HD = 16, 4, 64
CH = 512            # queries per core
NK = 1536           # key window per core (padded)
TPAD = 3072
EK = NE // 128      # 8 contraction tiles
NJT = NK // 128     # 12 key tiles
EPS = float(np.finfo(np.float32).eps)

f32 = mybir.dt.float32
f32r = mybir.dt.float32r
AF = mybir.ActivationFunctionType
OP = mybir.AluOpType
AX = mybir.AxisListType
SWAP_MASK = [m for i in range(0, 32, 2) for m in (i + 1, i)]
PAIRS = [(0, 4), (1, 5), (2, 6), (3, 7), (8, 12), (9, 13), (10, 14), (11, 15)]

_COMPILED = None


def _r(ap):
    return ap.bitcast(f32r)


def build_program(repeat=1):
    nc = bacc.Bacc("TRN2", target_bir_lowering=False, debug=False, num_devices=8)

    def din(name, shape, dt=f32):
        return nc.dram_tensor(name, shape, dt, kind="ExternalInput").ap()

    xt_d = din("xt", [128, EK * NK])          # x^T chunks e stacked along free
    wq_d = din("wqt", [128, EK * NE])         # wq^T chunks (pair-ordered cols)
    wk_d = din("wkt", [128, EK * NKV * HD])
    wv_d = din("wvt", [128, EK * NKV * HD])
    wg_d = din("wgt", [32, NKV])
    wp_d = din("wpt", [128, EK * NE])
    csa_d = din("csa", [128, NK])
    csb_d = din("csb", [128, NK])
    ve_d = din("ve", [128, NJT * NKV * HD])   # ve blocks j stacked along free
    vxi_d = din("vxi", [128, NJT * NKV])      # ones-column init (0 past T)
    mlo_d = din("mlo", [128, 4 * 512])
    mup_d = din("mup", [128, 128 + 3 * 512])
    out_d = nc.dram_tensor("out", [CH, NE], f32, kind="ExternalOutput").ap()

    ctx_vars = locals()
    with tile.TileContext(nc) as tc:
        for _ in range(repeat):
            _build(nc, tc, ctx_vars)

    nc.compile()
    return nc


def _build(nc, tc, d):
    from contextlib import ExitStack

    ctx = ExitStack()
    with ctx:
        # ---------------- persistent pools ----------------
        consts = ctx.enter_context(tc.tile_pool(name="consts", bufs=1))
        ktp = ctx.enter_context(tc.tile_pool(name="ktp", bufs=1))
        vxp = ctx.enter_context(tc.tile_pool(name="vxp", bufs=1))
        qtp = ctx.enter_context(tc.tile_pool(name="qtp", bufs=1))

        epst = consts.tile([128, 1], f32, tag="epst")
        nc.vector.memset(epst[:], 8.0 * EPS)

        ktall = ktp.tile([128, 2 * NK], f32r, tag="ktall")
        vx = vxp.tile([128, NJT * NKV * (HD + 1)], f32r, tag="vx")
        qt = qtp.tile([128, 8 * CH], f32r, tag="qt")

        last_tanh = [None]
        first_sqrt = [None]
        last_sqrt = [None]
        first_exp = [None]

        def rope_wide(pr_ap, out_ap, csa_ap, csb_ap, nrep, w, tmp):
            """pr_ap/out_ap: [128, nrep*w]; csa_ap/csb_ap: [128, w] APs.

            rope rotation + per-strip rmsnorm (incl. 1/sqrt(8) attn scale).
            15 instructions; 5 SBUF slots (sA..sE) via tag reuse. All gpsimd
            outputs sit at partition offset 0 of their own tile.
            """
            W = nrep * w
            csaB = csa_ap.unsqueeze(1).broadcast_to([128, nrep, w])
            csbB = csb_ap.unsqueeze(1).broadcast_to([128, nrep, w])
            prv = pr_ap.rearrange("p (r c) -> p r c", r=nrep)
            ta = tmp.tile([128, W], f32, tag="sA", name="ta")
            nc.vector.tensor_mul(ta[:].rearrange("p (r c) -> p r c", r=nrep), prv, csaB)
            tb = tmp.tile([128, W], f32, tag="sB", name="tb")
            nc.vector.tensor_mul(tb[:].rearrange("p (r c) -> p r c", r=nrep), prv, csbB)
            tbs = tmp.tile([128, W], f32, tag="sC", name="tbs")
            nc.vector.stream_shuffle(tbs[:], tb[:], SWAP_MASK)
            # rot overwrites the PSUM projection in place (PSUM operands mix
            # freely with SBUF bases; SB+SB operands must share base partition)
            nc.vector.tensor_add(pr_ap, ta[:], tbs[:])
            rot = pr_ap
            sq = tmp.tile([128, W], f32, tag="sA", name="sq")
            nc.scalar.activation(sq[:], rot, AF.Square)
            r2a = tmp.tile([1, W], f32, tag="sB", name="r2a")
            nc.gpsimd.tensor_reduce(r2a[:], sq[0:64, :], AX.C, OP.add)
            r2b = tmp.tile([1, W], f32, tag="sC", name="r2b")
            nc.gpsimd.tensor_reduce(r2b[:], sq[64:128, :], AX.C, OP.add)
            rqa = tmp.tile([1, W], f32, tag="sD", name="rqa")
            si = nc.scalar.activation(rqa[:], r2a[:], AF.Sqrt,
                                      bias=epst[0:1, 0:1], scale=0.125)
            if first_sqrt[0] is None:
                first_sqrt[0] = si
            rca = tmp.tile([1, W], f32, tag="sB", name="rca")
            nc.vector.reciprocal(rca[:], rqa[:])
            rqb = tmp.tile([1, W], f32, tag="sD", name="rqb")
            si = nc.scalar.activation(rqb[:], r2b[:], AF.Sqrt,
                                      bias=epst[0:1, 0:1], scale=0.125)
            last_sqrt[0] = si
            rcb = tmp.tile([1, W], f32, tag="sC", name="rcb")
            nc.vector.reciprocal(rcb[:], rqb[:])
            rsa = tmp.tile([64, W], f32, tag="sA", name="rsa")
            nc.gpsimd.partition_broadcast(rsa[:], rca[:], channels=64)
            rsb = tmp.tile([64, W], f32, tag="sB", name="rsb")
            nc.gpsimd.partition_broadcast(rsb[:], rcb[:], channels=64)
            nc.vector.tensor_mul(out_ap[0:64, :], rot[0:64, :], rsa[:])
            nc.vector.tensor_mul(out_ap[64:128, :], rot[64:128, :], rsb[:])

        # ======== phases 0-1 share xt ========
        with tc.tile_pool(name="xap", bufs=1) as xap:
            xt = xap.tile([128, EK * NK], f32r, tag="xt")
            nc.sync.dma_start(xt[:], _r(d["xt_d"][:]))
            csa = xap.tile([128, NK], f32, tag="csa")
            nc.sync.dma_start(csa[:], d["csa_d"][:])
            csb = xap.tile([128, NK], f32, tag="csb")
            nc.sync.dma_start(csb[:], d["csb_d"][:])

            # ================ phase 0: gates, K, V ================
            with (
                tc.tile_pool(name="p0", bufs=1) as p0,
                tc.tile_pool(name="tmp0", bufs=1) as tmp0,
                tc.tile_pool(name="psG", bufs=1, space="PSUM") as psG,
                tc.tile_pool(name="psK", bufs=1, space="PSUM") as psK,
                tc.tile_pool(name="psV", bufs=1, space="PSUM") as psV,
            ):
                wk = p0.tile([128, EK * NKV * HD], f32r, tag="wk")
                nc.sync.dma_start(wk[:], _r(d["wk_d"][:]))
                wv = p0.tile([128, EK * NKV * HD], f32r, tag="wv")
                nc.sync.dma_start(wv[:], _r(d["wv_d"][:]))
                wg = p0.tile([32, NKV], f32r, tag="wg")
                nc.sync.dma_start(wg[:], _r(d["wg_d"][:]))
                vet = p0.tile([128, NJT * NKV * HD], f32, tag="vet")
                nc.sync.dma_start(vet[:], d["ve_d"][:])
                vxi = p0.tile([128, NJT * NKV], f32, tag="vxi")
                nc.sync.dma_start(vxi[:], d["vxi_d"][:])
                vx_ones = vx[:].rearrange(
                    "p (j g c) -> p j g c", g=NKV, c=HD + 1)[:, :, :, HD]
                nc.vector.tensor_copy(vx_ones, vxi[:])

                # gates: one PSUM tile, 12 matmuls, 1 tanh, 1 add
                pg = psG.tile([128, NJT * NKV], f32, tag="pg")
                for j in range(NJT):
                    nc.tensor.matmul(pg[:, NKV * j:NKV * j + NKV],
                                     xt[0:32, 128 * j:128 * j + 128], wg[:],
                                     start=True, stop=True)
                g48 = p0.tile([128, NJT * NKV], f32, tag="g48")
                th = nc.scalar.activation(g48[:], pg[:], AF.Tanh, scale=0.5)
                last_tanh[0] = th
                gates = p0.tile([128, NJT * NKV], f32, tag="gates")
                nc.vector.tensor_scalar_add(gates[:], g48[:], 1.0)
                # gate value broadcast across each group's 64 hd columns
                gb = p0.tile([128, NJT * NKV * HD], f32, tag="gb")
                nc.vector.tensor_copy(
                    gb[:].rearrange("p (n c) -> p n c", c=HD),
                    gates[:].unsqueeze(2).broadcast_to([128, NJT * NKV, HD]))

                # K projection -> [128, 3072] PSUM, one wide rope+norm
                prK = psK.tile([128, 2 * NK], f32, tag="prK")
                for t in range(2):
                    for c in range(3):
                        o = NK * t + 512 * c
                        for e in range(EK):
                            nc.tensor.matmul(
                                prK[:, o:o + 512],
                                wk[:, 256 * e + 128 * t:256 * e + 128 * t + 128],
                                xt[:, NK * e + 512 * c:NK * e + 512 * c + 512],
                                start=(e == 0), stop=(e == EK - 1))
                rope_wide(prK[:], ktall[:], csa[:], csb[:], 2, NK, tmp0)

                # V projection (key-major) + gate/ve fold
                for m in range(6):
                    pv = psV.tile([128, 512], f32, tag="pv", name="pv")
                    for r in range(2):
                        j = 2 * m + r
                        for e in range(EK):
                            nc.tensor.matmul(
                                pv[:, 256 * r:256 * r + 256],
                                xt[:, NK * e + 128 * j:NK * e + 128 * j + 128],
                                wv[:, 256 * e:256 * e + 256],
                                start=(e == 0), stop=(e == EK - 1))
                    j0 = 2 * m
                    gv = p0.tile([128, 512], f32, tag="gv", name="gv", bufs=2)
                    nc.vector.tensor_mul(gv[:], vet[:, 256 * j0:256 * j0 + 512],
                                         gb[:, 256 * j0:256 * j0 + 512])
                    vxv = vx[:].rearrange("p (j g c) -> p j g c",
                                          g=NKV, c=HD + 1)[:, j0:j0 + 2, :, 0:HD]
                    nc.vector.tensor_add(vxv, gv[:].rearrange(
                        "p (j g c) -> p j g c", j=2, g=NKV),
                        pv[:].rearrange("p (j g c) -> p j g c", j=2, g=NKV))

            # ================ phase 1: Q projection + wide rope ================
            with (
                tc.tile_pool(name="p1", bufs=1) as p1,
                tc.tile_pool(name="tmp1", bufs=1) as tmp1,
                tc.tile_pool(name="psQ", bufs=1, space="PSUM") as psQ,
            ):
                wq = p1.tile([128, EK * 512], f32r, tag="wq")
                wq_v = wq[:].rearrange("p (e c) -> p e c", e=EK)
                wqd_v = _r(d["wq_d"][:]).rearrange("p (e c) -> p e c", e=EK)
                prQ = psQ.tile([128, 8 * CH], f32, tag="prQ")
                for half in range(2):
                    nc.sync.dma_start(wq_v, wqd_v[:, :, 512 * half:512 * half + 512])
                    for p4 in range(4):
                        p = 4 * half + p4
                        for e in range(EK):
                            nc.tensor.matmul(
                                prQ[:, 512 * p:512 * p + 512],
                                wq[:, 512 * e + 128 * p4:512 * e + 128 * p4 + 128],
                                xt[:, NK * e:NK * e + 512],
                                start=(e == 0), stop=(e == EK - 1))
                rope_wide(prQ[:], qt[:], csa[:, 0:CH], csb[:, 0:CH],
                          8, CH, tmp1)

        # ================ phase 2: attention ================
        ytp = ctx.enter_context(tc.tile_pool(name="ytp", bufs=1))
        ytall = ytp.tile([128, EK * CH], f32r, tag="ytall")
        with (
            tc.tile_pool(name="aux2", bufs=1) as aux2,
            tc.tile_pool(name="ptp", bufs=1) as ptp,
            tc.tile_pool(name="tmpN", bufs=2) as tmpN,
            tc.tile_pool(name="psS", bufs=1, space="PSUM") as psS,
            tc.tile_pool(name="psO", bufs=2, space="PSUM") as psO,
        ):
            mlo = aux2.tile([128, 4 * 512], f32, tag="mlo")
            nc.sync.dma_start(mlo[:], d["mlo_d"][:])
            mup = aux2.tile([128, 128 + 3 * 512], f32, tag="mup")
            nc.sync.dma_start(mup[:], d["mup_d"][:])
            mupB = mup[:, 0:128].unsqueeze(1).broadcast_to([128, 2, 128])
            mloT = mlo[:, 0:1536].rearrange("p (j c) -> p j c", j=3).unsqueeze(
                2).broadcast_to([128, 3, 2, 512])
            mupT = mup[:, 128:128 + 1536].rearrange(
                "p (j c) -> p j c", j=3).unsqueeze(2).broadcast_to([128, 3, 2, 512])

            for p in range(8):
                hA, hB = PAIRS[p]
                kto = NK * ((hA // 4) // 2)
                ots = []
                for idx, h in enumerate((hA, hB)):
                    ot = psO.tile([HD + 1, CH], f32, tag="ot", name=f"ot{h}")
                    ots.append(ot)
                for tr in range(4):
                    st = psS.tile([128, 3072], f32, tag="st", name="st")
                    for jj in range(3):
                        jt = 3 * tr + jj
                        for s in range(2):
                            nc.tensor.matmul(
                                st[:, 1024 * jj + 512 * s:1024 * jj + 512 * s + 512],
                                ktall[64 * s:64 * s + 64,
                                      kto + 128 * jt:kto + 128 * jt + 128],
                                qt[64 * s:64 * s + 64, 512 * p:512 * p + 512],
                                start=True, stop=True)
                    pt = ptp.tile([128, 3072], f32r, tag="pt", name="pt")
                    ei = nc.scalar.activation(pt[:], st[:], AF.Exp)
                    if first_exp[0] is None:
                        first_exp[0] = ei
                    ptv = pt[:].rearrange("q (j s c) -> q j s c", j=3, s=2)
                    if tr == 0:
                        nc.vector.tensor_mul(ptv, ptv, mloT)
                    elif tr == 1:
                        pvv = ptv[:, 0, :, :]
                        mloB = mlo[:, 1536:2048].unsqueeze(1).broadcast_to(
                            [128, 2, 512])
                        nc.vector.tensor_mul(pvv, pvv, mloB)
                    elif tr == 2:
                        pvv = ptv[:, 2, :, 0:128]
                        nc.vector.tensor_mul(pvv, pvv, mupB)
                    else:
                        nc.vector.tensor_mul(ptv, ptv, mupT)
                    for jj in range(3):
                        jt = 3 * tr + jj
                        iw0 = 128 * max(0, jt - 8)
                        for idx, h in enumerate((hA, hB)):
                            g = h // 4
                            nc.tensor.matmul(
                                ots[idx][:, iw0:CH],
                                vx[:, 260 * jt + 65 * g:260 * jt + 65 * g + 65],
                                pt[:, 1024 * jj + 512 * idx + iw0:
                                   1024 * jj + 512 * idx + CH],
                                start=(jt == 0), stop=(jt == NJT - 1))
                # normalization: per-head reciprocal of the ones-row, bcast, mul
                rsbs = []
                for idx in range(2):
                    rs1 = tmpN.tile([1, CH], f32, tag="rs1", name=f"rs1_{p}_{idx}")
                    nc.vector.reciprocal(rs1[:], ots[idx][HD:HD + 1, :])
                    rsbx = tmpN.tile([64, CH], f32, tag="rsbx", name=f"rsb_{p}_{idx}")
                    nc.gpsimd.partition_broadcast(rsbx[:], rs1[:], channels=64)
                    rsbs.append(rsbx)
                for idx, h in enumerate((hA, hB)):
                    f, rr = h // 2, h % 2
                    nc.vector.tensor_mul(
                        ytall[64 * rr:64 * rr + 64, 512 * f:512 * f + 512],
                        ots[idx][0:HD, :], rsbs[idx][:])

        # ACT table grouping: tanh -> sqrt -> exp
        if first_sqrt[0] is not None and last_tanh[0] is not None:
            add_dep_helper(first_sqrt[0].ins, last_tanh[0].ins, sync=False,
                           reason="group ACT tanh before sqrt")
        if first_exp[0] is not None and last_sqrt[0] is not None:
            add_dep_helper(first_exp[0].ins, last_sqrt[0].ins, sync=False,
                           reason="group ACT sqrt before exp")

        # ================ phase 3: output projection ================
        with (
            tc.tile_pool(name="p3", bufs=1) as p3,
            tc.tile_pool(name="pop", bufs=2) as pop,
            tc.tile_pool(name="psP", bufs=2, space="PSUM") as psP,
        ):
            wp = p3.tile([128, EK * NE], f32r, tag="wp")
            nc.sync.dma_start(wp[:], _r(d["wp_d"][:]))
            for it in range(4):
                for half in range(2):
                    pp = psP.tile([128, 512], f32, tag="pp", name="pp")
                    for f in range(EK):
                        nc.tensor.matmul(
                            pp[:],
                            ytall[:, 512 * f + 128 * it:512 * f + 128 * it + 128],
                            wp[:, NE * f + 512 * half:NE * f + 512 * half + 512],
                            start=(f == 0), stop=(f == EK - 1))
                    po = pop.tile([128, 512], f32, tag="po", name="po")
                    nc.scalar.copy(po[:], pp[:])
                    nc.sync.dma_start(
                        d["out_d"][128 * it:128 * it + 128,
                                   512 * half:512 * half + 512],
                        po[:])


# ---------------- host prep ----------------

def host_prep(inputs):
    x = np.asarray(inputs["x"], np.float32)
    ve = np.asarray(inputs["ve"], np.float32)
    cos = np.asarray(inputs["cos"], np.float32)
    sin = np.asarray(inputs["sin"], np.float32)
    wq = np.asarray(inputs["wq"], np.float32)
    wk = np.asarray(inputs["wk"], np.float32)
    wv = np.asarray(inputs["wv"], np.float32)
    wproj = np.asarray(inputs["wproj"], np.float32)
    wgate = np.asarray(inputs["wgate"], np.float32)

    def rope_perm(nh):
        idx = np.empty(nh * 64, np.int64)
        for h in range(nh):
            for dd in range(32):
                for half in range(2):
                    idx[h * 64 + 2 * dd + half] = h * 64 + 32 * half + dd
        return idx

    def chunks_h(a):
        # [1024, F] -> [128, 8*F] (contraction chunks stacked along free dim)
        return np.ascontiguousarray(np.hstack(list(a.reshape(8, 128, -1))))

    XT = np.zeros((B, NE, TPAD), np.float32)
    XT[:, :, :T] = x.transpose(0, 2, 1)
    VEP = np.zeros((B, TPAD, NKV * HD), np.float32)
    VEP[:, :T] = ve

    wq_perm = wq.T[:, rope_perm(NH)]
    cols = []
    for hA, hB in PAIRS:
        cols.extend(range(64 * hA, 64 * hA + 64))
        cols.extend(range(64 * hB, 64 * hB + 64))
    wq_t = chunks_h(wq_perm[:, cols])
    wk_t = chunks_h(wk.T[:, rope_perm(NKV)])
    wv_t = chunks_h(wv.T)
    wp_t = chunks_h(wproj.T)
    wg_t = np.ascontiguousarray(wgate.T)

    cosT = np.zeros((32, TPAD), np.float32)
    sinT = np.zeros((32, TPAD), np.float32)
    cosT[:, :T] = cos[0, :, 0, :].T
    sinT[:, :T] = sin[0, :, 0, :].T
    csa64 = np.empty((64, TPAD), np.float32)
    csb64 = np.empty((64, TPAD), np.float32)
    csa64[0::2] = cosT
    csa64[1::2] = cosT
    csb64[0::2] = -sinT
    csb64[1::2] = sinT
    CSA = np.concatenate([csa64, csa64], 0)
    CSB = np.concatenate([csb64, csb64], 0)

    jj = np.arange(128)[:, None]
    ii = np.arange(128)[None, :]
    tri = (ii <= jj)
    triu = (ii >= jj)
    # full-width masks for jt<=3: [ones(128*jt) | tri | zeros]
    mlo = np.zeros((128, 4, 512), np.float32)
    for jt in range(4):
        mlo[:, jt, :128 * jt] = 1.0
        mlo[:, jt, 128 * jt:128 * (jt + 1)] = tri
    mlo = np.ascontiguousarray(mlo.reshape(128, 4 * 512))
    # mup: [plain triu | full-width masks for jt=9,10,11: zeros|triu|ones]
    mup = np.zeros((128, 128 + 3 * 512), np.float32)
    mup[:, 0:128] = triu
    for idx, jt in enumerate((9, 10, 11)):
        i = jt - 8
        o = 128 + 512 * idx
        mup[:, o + 128 * i:o + 128 * (i + 1)] = triu
        mup[:, o + 128 * (i + 1):o + 512] = 1.0

    in_maps = []
    for c in range(8):
        b, ci = c // 4, c % 4
        q0 = CH * ci
        xw = XT[b][:, q0:q0 + NK]                      # [1024, 1536]
        vew = VEP[b][q0:q0 + NK]                       # [1536, 256]
        ve_t = np.ascontiguousarray(
            np.hstack(list(vew.reshape(NJT, 128, NKV * HD))))
        # ones-init: 1.0 where key position < T else 0, [128, 12*4]
        kpos = (q0 + 128 * np.arange(NJT)[None, :, None]
                + np.arange(128)[:, None, None])
        vxi = (np.broadcast_to(kpos, (128, NJT, NKV)) < T).astype(np.float32)
        vxi = np.ascontiguousarray(vxi.reshape(128, NJT * NKV))
        in_maps.append({
            "xt": chunks_h(xw),
            "wqt": wq_t, "wkt": wk_t, "wvt": wv_t, "wgt": wg_t, "wpt": wp_t,
            "csa": np.ascontiguousarray(CSA[:, q0:q0 + NK]),
            "csb": np.ascontiguousarray(CSB[:, q0:q0 + NK]),
            "ve": ve_t,
            "vxi": vxi,
            "mlo": mlo, "mup": mup,
        })
    return in_maps


def kernel(**inputs):
    global _COMPILED
    if _COMPILED is None:
        _COMPILED = build_program()
    nc = _COMPILED
    in_maps = host_prep(inputs)

    from concourse.bass_utils import run_bass_kernel_spmd
    res = run_bass_kernel_spmd(nc, in_maps, list(range(8)))

    out = np.empty((B, T, NE), np.float32)
    for c in range(8):
        b, ci = c // 4, c % 4
        out[b, CH * ci:CH * ci + CH] = res.results[c]["out"]
    return out
